# revision 18
# baseline (speedup 1.0000x reference)
"""Trainium2 Bass kernel for HDGradientCompressionLayer forward.

Reference computation: y = einsum("bsd,df->bsf", x, W) + b
  x: (4, 4096, 1024) f32, W: (1024, 1024) f32, b: (1024,) f32.

Strategy (data-parallel across 8 cores, per sharding hint):
  Flatten x to (16384, 1024); each core gets 2048 rows (= 16 rowblocks
  of 128).  All layout work happens on the HOST so the device does pure
  HWDGE copy DMAs and the PE does only the 256 bf16 matmuls:
    - host casts x/W to bf16, pre-transposes each core's x shard to
      xT [d, m], packs the first 512 xT columns with W ("wxa") so one
      DMA per d-block delivers both matmul operands, and pre-broadcasts
      the f32 bias (a plain HWDGE copy instead of a slow SWDGE replicate
      that would starve the load queue),
    - loads are split across BOTH HWDGE rings (SP + ACT); the ACT ring
      is held back ~1.5us by two filler copies so the first two k-tiles
      drain the SP ring SOLO at full bandwidth — their completion
      semaphores otherwise jitter by +2us under concurrent traffic,
    - device: rowblock groups of (4,4,4,3,1) (PSUM-bank limited),
      k-outer accumulation psum[m,f] += xT[k][:,m-slice].T @ W[k][:,f];
      the 1-rowblock final group accumulates bank-major so bank n0
      evicts/stores while bank n1 is still accumulating,
    - DVE adds the bias during PSUM->SBUF eviction; the final bank is
      evicted in halves with 128KB stores alternating across both HWDGE
      rings to keep the post-matmul tail short.
"""

import os

import numpy as np

import concourse.bass as bass
import concourse.bacc as bacc
import concourse.tile as tile
from concourse import mybir
from concourse.bass_utils import run_bass_kernel_spmd

N_CORES = 8
B, S, D = 4, 4096, 1024
F = 1024
ROWS_TOTAL = B * S            # 16384
ROWS = ROWS_TOTAL // N_CORES  # 2048 per core
P = 128
NSPLIT = 512                  # one PSUM bank of f32
KB = D // P                   # 8 contraction blocks
RB = ROWS // P                # 16 rowblocks per core
GROUPS = (4, 4, 4, 3, 1)      # rowblocks per PSUM group (<=4: 8 banks)
MA = 512                      # xT columns packed with W into wxa
MB = 512                      # xT columns in wxb
MR = ROWS - MA - MB           # 1024 xT columns in xr

# wxa row layout: [ x m0:512 | W f0:1024 ]
XA0 = 0                       # x cols m 0:512
WN0 = MA                      # W cols 0:512
WN1 = MA + NSPLIT             # W cols 512:1024
WXA_COLS = F + MA
K0SPLIT = MA + NSPLIT         # k0 chunk1 = [0:1024] (x cols + W n0)

_BF16 = mybir.dt.np(mybir.dt.bfloat16)


def build_nc() -> bass.Bass:
    nc = bacc.Bacc("TRN2", target_bir_lowering=False, debug=False)
    wxa = nc.dram_tensor("wxa", [D, WXA_COLS], mybir.dt.bfloat16, kind="ExternalInput").ap()
    wxb = nc.dram_tensor("wxb", [D, MB], mybir.dt.bfloat16, kind="ExternalInput").ap()
    xr = nc.dram_tensor("xr", [D, MR], mybir.dt.bfloat16, kind="ExternalInput").ap()
    bb = nc.dram_tensor("bb", [P, F], mybir.dt.float32, kind="ExternalInput").ap()
    y = nc.dram_tensor("y", [ROWS, F], mybir.dt.float32, kind="ExternalOutput").ap()

    with tile.TileContext(nc) as tc:
        with tc.tile_pool(name="const", bufs=1) as const, \
             tc.tile_pool(name="ap", bufs=1) as apool, \
             tc.tile_pool(name="bp", bufs=1) as bpool, \
             tc.tile_pool(name="rp", bufs=1) as rpool, \
             tc.tile_pool(name="yp", bufs=1) as yp, \
             tc.tile_pool(name="scp", bufs=1) as scp, \
             tc.tile_pool(name="psp", bufs=1, space="PSUM") as psp:

            # HAM warmup: full-array matmuls on a zeroed tile (1-partition
            # ones do NOT register as PE activity) bridge the window from
            # the preamble to the first data arrival.
            warm = const.tile([P, NSPLIT], mybir.dt.bfloat16)
            nc.vector.memset(warm[:], 0.0)
            warm_ps = psp.tile([P, NSPLIT], mybir.dt.float32, tag="ps_0_0", bufs=1)
            for _ in range(6):
                nc.tensor.matmul(warm_ps[:], warm[:, 0:P], warm[:],
                                 start=True, stop=True, skip_group_check=True)

            # Loads split across both HWDGE rings (SP=sync, ACT=scalar) in
            # consumption order; the k0 tile is split so the first matmul's
            # 160KB chunk and the rest arrive in parallel on the two rings.
            wxa_t = [apool.tile([P, WXA_COLS], mybir.dt.bfloat16, name=f"wxa{k}", tag=f"wxa{k}")
                     for k in range(KB)]
            wxb_t = [bpool.tile([P, MB], mybir.dt.bfloat16, name=f"wxb{k}", tag=f"wxb{k}")
                     for k in range(KB)]
            xr_all = rpool.tile([P, KB, MR], mybir.dt.bfloat16, name="xr_all", tag="xr_all")
            b_bc = const.tile([P, F], mybir.dt.float32)

            # SP ring: k0, k1 (solo-drain window), k3, k5, k7, then xr.
            for k in (0, 1, 3, 5, 7):
                nc.sync.dma_start(wxa_t[k][:], wxa[k * P:(k + 1) * P, :])
            nc.sync.dma_start(xr_all[:], xr.rearrange("(k p) m -> p k m", p=P))
            # ACT ring: two filler copies delay its first DMA ~1.5us so the
            # SP ring's k0/k1 tiles drain solo; then k2/k4/k6, bias, wxb.
            act_stall = const.tile([P, NSPLIT], mybir.dt.bfloat16)
            nc.scalar.copy(act_stall[:], warm[:])
            nc.scalar.copy(act_stall[:], warm[:])
            for k in (2, 4, 6):
                nc.scalar.dma_start(wxa_t[k][:], wxa[k * P:(k + 1) * P, :])
            nc.scalar.dma_start(b_bc[:], bb[:, :])
            for k in range(KB):
                nc.scalar.dma_start(wxb_t[k][:], wxb[k * P:(k + 1) * P, :])

            def xslice(k: int, rb: int):
                m0 = rb * P
                if m0 < MA:
                    return wxa_t[k][:, m0:m0 + P]
                if m0 < MA + MB:
                    return wxb_t[k][:, m0 - MA:m0 - MA + P]
                return xr_all[:, k, m0 - MA - MB:m0 - MA - MB + P]

            def wslice(k: int, n: int):
                base = WN0 if n == 0 else WN1
                return wxa_t[k][:, base:base + NSPLIT]

            rb0 = 0
            for gi, gsz in enumerate(GROUPS):
                # The 1-rowblock final group takes the ps_3 tags (last
                # evicted two groups ago) so its k=0 matmul never waits.
                joff = 3 if gsz == 1 else 0
                ps = [[psp.tile([P, NSPLIT], mybir.dt.float32, name=f"ps_{j + joff}_{n}",
                                tag=f"ps_{j + joff}_{n}", bufs=1)
                       for n in range(2)] for j in range(gsz)]
                if gsz == 1:
                    # Final rowblock: bank-major accumulation so bank n0 is
                    # complete (and evicting/storing) while n1 accumulates;
                    # after the last matmul only one bank remains to drain.
                    rb = rb0
                    y_sb = yp.tile([P, F], mybir.dt.float32, name="ysb", tag="ysb", bufs=6)
                    for n in range(2):
                        for k in range(KB):
                            nc.tensor.matmul(
                                ps[0][n][:], xslice(k, rb), wslice(k, n),
                                start=(k == 0), stop=(k == KB - 1),
                            )
                        for h in range(2):
                            c0 = n * NSPLIT + h * (NSPLIT // 2)
                            c1 = c0 + NSPLIT // 2
                            nc.vector.tensor_add(
                                y_sb[:, c0:c1],
                                ps[0][n][:, h * (NSPLIT // 2):(h + 1) * (NSPLIT // 2)],
                                b_bc[:, c0:c1],
                            )
                            eng = nc.scalar if h == 0 else nc.sync
                            eng.dma_start(y[rb * P:(rb + 1) * P, c0:c1], y_sb[:, c0:c1])
                    rb0 += gsz
                    continue
                for k in range(KB):
                    for j in range(gsz):
                        xs = xslice(k, rb0 + j)
                        for n in range(2):
                            nc.tensor.matmul(
                                ps[j][n][:], xs, wslice(k, n),
                                start=(k == 0), stop=(k == KB - 1),
                            )
                if rb0 + gsz <= 12:
                    # Early groups: DVE bias-adds drain the n0 banks while
                    # ACT copies free the n1 banks in parallel (halving the
                    # PSUM WAR window at the group boundary); the n1 bias
                    # is added later from SBUF, whose deadline is only the
                    # store, not the next group's start=True matmuls.
                    y_sbs = [yp.tile([P, F], mybir.dt.float32, name="ysb", tag="ysb", bufs=6)
                             for _ in range(gsz)]
                    for j in range(gsz):
                        nc.vector.tensor_add(
                            y_sbs[j][:, 0:NSPLIT], ps[j][0][:], b_bc[:, 0:NSPLIT])
                    scs = []
                    for j in range(gsz):
                        sc = scp.tile([P, NSPLIT], mybir.dt.float32, name="sc", tag="sc", bufs=3)
                        nc.scalar.copy(sc[:], ps[j][1][:])
                        scs.append(sc)
                    for j in range(gsz):
                        nc.vector.tensor_add(
                            y_sbs[j][:, NSPLIT:], scs[j][:], b_bc[:, NSPLIT:])
                        nc.scalar.dma_start(y[(rb0 + j) * P:(rb0 + j + 1) * P, :], y_sbs[j][:])
                else:
                    for j in range(gsz):
                        rb = rb0 + j
                        y_sb = yp.tile([P, F], mybir.dt.float32, name="ysb", tag="ysb", bufs=6)
                        for n in range(2):
                            nc.vector.tensor_add(
                                y_sb[:, n * NSPLIT:(n + 1) * NSPLIT],
                                ps[j][n][:],
                                b_bc[:, n * NSPLIT:(n + 1) * NSPLIT],
                            )
                            # End-of-kernel stores are bandwidth-bound:
                            # drain each bank on its own ring immediately.
                            eng = nc.scalar if n == 0 else nc.sync
                            eng.dma_start(y[rb * P:(rb + 1) * P, n * NSPLIT:(n + 1) * NSPLIT],
                                          y_sb[:, n * NSPLIT:(n + 1) * NSPLIT])
                rb0 += gsz

    nc.compile()
    return nc


_NC_CACHE: dict[str, bass.Bass] = {}


def _get_nc() -> bass.Bass:
    if "nc" not in _NC_CACHE:
        _NC_CACHE["nc"] = build_nc()
    return _NC_CACHE["nc"]


def _run(in_maps, trace: bool = False):
    nc = _get_nc()
    return run_bass_kernel_spmd(nc, in_maps, list(range(N_CORES)), trace=trace)


def make_in_maps(x: np.ndarray, W: np.ndarray, b: np.ndarray):
    x = np.ascontiguousarray(np.asarray(x, dtype=np.float32)).reshape(ROWS_TOTAL, D)
    W_bf = np.asarray(W, dtype=np.float32).astype(_BF16)
    b = np.asarray(b, dtype=np.float32)
    bb = np.ascontiguousarray(np.broadcast_to(b[None, :], (P, F)))
    in_maps = []
    for c in range(N_CORES):
        shard_bf = x[c * ROWS:(c + 1) * ROWS].astype(_BF16)  # [2048, 1024]
        wxa = np.empty((D, WXA_COLS), dtype=_BF16)
        wxa[:, :MA] = shard_bf[:MA].T
        wxa[:, MA:] = W_bf
        wxb = np.ascontiguousarray(shard_bf[MA:MA + MB].T)
        xr = np.ascontiguousarray(shard_bf[MA + MB:].T)
        in_maps.append({"wxa": wxa, "wxb": wxb, "xr": xr, "bb": bb})
    return in_maps


def kernel(x: np.ndarray, W: np.ndarray, b: np.ndarray) -> np.ndarray:
    in_maps = make_in_maps(x, W, b)
    res = _run(in_maps, trace=bool(int(os.environ.get("BASS_KERNEL_TRACE", "0"))))
    y = np.concatenate([res.results[c]["y"] for c in range(N_CORES)], axis=0)
    return y.reshape(B, S, F)


# revision 19
# speedup vs baseline: 1.0920x; 1.0920x over previous
"""Trainium2 Bass kernel for HDGradientCompressionLayer forward.

Reference computation: y = einsum("bsd,df->bsf", x, W) + b
  x: (4, 4096, 1024) f32, W: (1024, 1024) f32, b: (1024,) f32.

Strategy (data-parallel across 8 cores, per sharding hint):
  Flatten x to (16384, 1024); each core gets 2048 rows (= 16 rowblocks
  of 128).  All layout work happens on the HOST so the device does pure
  HWDGE copy DMAs and the PE does only the 256 bf16 matmuls:
    - host casts x/W to bf16, pre-transposes each core's x shard to
      xT [d, m], packs the first 512 xT columns with W ("wxa") so one
      DMA per d-block delivers both matmul operands, and pre-broadcasts
      the f32 bias (a plain HWDGE copy instead of a slow SWDGE replicate
      that would starve the load queue),
    - loads are split across BOTH HWDGE rings (SP + ACT); the ACT ring
      is held back ~1.5us by two filler copies so the first two k-tiles
      drain the SP ring SOLO at full bandwidth — their completion
      semaphores otherwise jitter by +2us under concurrent traffic,
    - device: rowblock groups of (4,4,4,3,1) (PSUM-bank limited),
      k-outer accumulation psum[m,f] += xT[k][:,m-slice].T @ W[k][:,f];
      the 1-rowblock final group accumulates bank-major so bank n0
      evicts/stores while bank n1 is still accumulating,
    - DVE adds the bias during PSUM->SBUF eviction; the final bank is
      evicted in halves with 128KB stores alternating across both HWDGE
      rings to keep the post-matmul tail short.
"""

import os

import numpy as np

import concourse.bass as bass
import concourse.bacc as bacc
import concourse.tile as tile
from concourse import mybir
from concourse.bass_utils import run_bass_kernel_spmd

N_CORES = 8
B, S, D = 4, 4096, 1024
F = 1024
ROWS_TOTAL = B * S            # 16384
ROWS = ROWS_TOTAL // N_CORES  # 2048 per core
P = 128
NSPLIT = 512                  # one PSUM bank of f32
KB = D // P                   # 8 contraction blocks
RB = ROWS // P                # 16 rowblocks per core
GROUPS = (4, 4, 4, 3, 1)      # rowblocks per PSUM group (<=4: 8 banks)
MA = 512                      # xT columns packed with W into wxa
MB = 512                      # xT columns in wxb
MR = ROWS - MA - MB           # 1024 xT columns in xr

# wxa row layout: [ x m0:512 | W f0:1024 ]
XA0 = 0                       # x cols m 0:512
WN0 = MA                      # W cols 0:512
WN1 = MA + NSPLIT             # W cols 512:1024
WXA_COLS = F + MA
K0SPLIT = MA + NSPLIT         # k0 chunk1 = [0:1024] (x cols + W n0)

_BF16 = mybir.dt.np(mybir.dt.bfloat16)


def build_nc() -> bass.Bass:
    nc = bacc.Bacc("TRN2", target_bir_lowering=False, debug=False)
    wxa = nc.dram_tensor("wxa", [D, WXA_COLS], mybir.dt.bfloat16, kind="ExternalInput").ap()
    wxb = nc.dram_tensor("wxb", [D, MB], mybir.dt.bfloat16, kind="ExternalInput").ap()
    xr = nc.dram_tensor("xr", [D, MR], mybir.dt.bfloat16, kind="ExternalInput").ap()
    bb = nc.dram_tensor("bb", [P, F], mybir.dt.float32, kind="ExternalInput").ap()
    y = nc.dram_tensor("y", [ROWS, F], mybir.dt.float32, kind="ExternalOutput").ap()

    with tile.TileContext(nc) as tc:
        with tc.tile_pool(name="const", bufs=1) as const, \
             tc.tile_pool(name="ap", bufs=1) as apool, \
             tc.tile_pool(name="bp", bufs=1) as bpool, \
             tc.tile_pool(name="rp", bufs=1) as rpool, \
             tc.tile_pool(name="yp", bufs=1) as yp, \
             tc.tile_pool(name="psp", bufs=1, space="PSUM") as psp:

            # HAM warmup: full-array matmuls on a zeroed tile (1-partition
            # ones do NOT register as PE activity) bridge the window from
            # the preamble to the first data arrival.
            warm = const.tile([P, NSPLIT], mybir.dt.bfloat16)
            nc.vector.memset(warm[:], 0.0)
            warm_ps = psp.tile([P, NSPLIT], mybir.dt.float32, tag="ps_0_0", bufs=1)
            for _ in range(6):
                nc.tensor.matmul(warm_ps[:], warm[:, 0:P], warm[:],
                                 start=True, stop=True, skip_group_check=True)

            # Loads split across both HWDGE rings (SP=sync, ACT=scalar) in
            # consumption order; the k0 tile is split so the first matmul's
            # 160KB chunk and the rest arrive in parallel on the two rings.
            wxa_t = [apool.tile([P, WXA_COLS], mybir.dt.bfloat16, name=f"wxa{k}", tag=f"wxa{k}")
                     for k in range(KB)]
            wxb_t = [bpool.tile([P, MB], mybir.dt.bfloat16, name=f"wxb{k}", tag=f"wxb{k}")
                     for k in range(KB)]
            xr_all = rpool.tile([P, KB, MR], mybir.dt.bfloat16, name="xr_all", tag="xr_all")
            b_bc = const.tile([P, F], mybir.dt.float32)

            # SP ring: k0, k1 (solo-drain window), k3, k5, k7, then xr.
            for k in (0, 1, 3, 5, 7):
                nc.sync.dma_start(wxa_t[k][:], wxa[k * P:(k + 1) * P, :])
            nc.sync.dma_start(xr_all[:], xr.rearrange("(k p) m -> p k m", p=P))
            # ACT ring: two filler copies delay its first DMA ~1.5us so the
            # SP ring's k0/k1 tiles drain solo; then k2/k4/k6, bias, wxb.
            act_stall = const.tile([P, NSPLIT], mybir.dt.bfloat16)
            nc.scalar.copy(act_stall[:], warm[:])
            nc.scalar.copy(act_stall[:], warm[:])
            for k in (2, 4, 6):
                nc.scalar.dma_start(wxa_t[k][:], wxa[k * P:(k + 1) * P, :])
            nc.scalar.dma_start(b_bc[:], bb[:, :])
            for k in range(KB):
                nc.scalar.dma_start(wxb_t[k][:], wxb[k * P:(k + 1) * P, :])

            def xslice(k: int, rb: int):
                m0 = rb * P
                if m0 < MA:
                    return wxa_t[k][:, m0:m0 + P]
                if m0 < MA + MB:
                    return wxb_t[k][:, m0 - MA:m0 - MA + P]
                return xr_all[:, k, m0 - MA - MB:m0 - MA - MB + P]

            def wslice(k: int, n: int):
                base = WN0 if n == 0 else WN1
                return wxa_t[k][:, base:base + NSPLIT]

            rb0 = 0
            for gi, gsz in enumerate(GROUPS):
                # The 1-rowblock final group takes the ps_3 tags (last
                # evicted two groups ago) so its k=0 matmul never waits.
                joff = 3 if gsz == 1 else 0
                ps = [[psp.tile([P, NSPLIT], mybir.dt.float32, name=f"ps_{j + joff}_{n}",
                                tag=f"ps_{j + joff}_{n}", bufs=1)
                       for n in range(2)] for j in range(gsz)]
                if gsz == 1:
                    # Final rowblock: bank-major accumulation so bank n0 is
                    # complete (and evicting/storing) while n1 accumulates;
                    # after the last matmul only one bank remains to drain.
                    rb = rb0
                    y_sb = yp.tile([P, F], mybir.dt.float32, name="ysb", tag="ysb", bufs=6)
                    for n in range(2):
                        for k in range(KB):
                            nc.tensor.matmul(
                                ps[0][n][:], xslice(k, rb), wslice(k, n),
                                start=(k == 0), stop=(k == KB - 1),
                            )
                        for h in range(2):
                            c0 = n * NSPLIT + h * (NSPLIT // 2)
                            c1 = c0 + NSPLIT // 2
                            nc.vector.tensor_add(
                                y_sb[:, c0:c1],
                                ps[0][n][:, h * (NSPLIT // 2):(h + 1) * (NSPLIT // 2)],
                                b_bc[:, c0:c1],
                            )
                            eng = nc.scalar if h == 0 else nc.sync
                            eng.dma_start(y[rb * P:(rb + 1) * P, c0:c1], y_sb[:, c0:c1])
                    rb0 += gsz
                    continue
                for k in range(KB):
                    for j in range(gsz):
                        xs = xslice(k, rb0 + j)
                        for n in range(2):
                            nc.tensor.matmul(
                                ps[j][n][:], xs, wslice(k, n),
                                start=(k == 0), stop=(k == KB - 1),
                            )
                for j in range(gsz):
                    rb = rb0 + j
                    y_sb = yp.tile([P, F], mybir.dt.float32, name="ysb", tag="ysb", bufs=6)
                    late = rb >= 12
                    for n in range(2):
                        nc.vector.tensor_add(
                            y_sb[:, n * NSPLIT:(n + 1) * NSPLIT],
                            ps[j][n][:],
                            b_bc[:, n * NSPLIT:(n + 1) * NSPLIT],
                        )
                        if late:
                            # End-of-kernel stores are bandwidth-bound:
                            # drain each bank on its own ring immediately.
                            eng = nc.scalar if n == 0 else nc.sync
                            eng.dma_start(y[rb * P:(rb + 1) * P, n * NSPLIT:(n + 1) * NSPLIT],
                                          y_sb[:, n * NSPLIT:(n + 1) * NSPLIT])
                    if not late:
                        nc.scalar.dma_start(y[rb * P:(rb + 1) * P, :], y_sb[:])
                rb0 += gsz

    nc.compile()
    return nc


_NC_CACHE: dict[str, bass.Bass] = {}


def _get_nc() -> bass.Bass:
    if "nc" not in _NC_CACHE:
        _NC_CACHE["nc"] = build_nc()
    return _NC_CACHE["nc"]


def _run(in_maps, trace: bool = False):
    nc = _get_nc()
    return run_bass_kernel_spmd(nc, in_maps, list(range(N_CORES)), trace=trace)


def make_in_maps(x: np.ndarray, W: np.ndarray, b: np.ndarray):
    x = np.ascontiguousarray(np.asarray(x, dtype=np.float32)).reshape(ROWS_TOTAL, D)
    W_bf = np.asarray(W, dtype=np.float32).astype(_BF16)
    b = np.asarray(b, dtype=np.float32)
    bb = np.ascontiguousarray(np.broadcast_to(b[None, :], (P, F)))
    in_maps = []
    for c in range(N_CORES):
        shard_bf = x[c * ROWS:(c + 1) * ROWS].astype(_BF16)  # [2048, 1024]
        wxa = np.empty((D, WXA_COLS), dtype=_BF16)
        wxa[:, :MA] = shard_bf[:MA].T
        wxa[:, MA:] = W_bf
        wxb = np.ascontiguousarray(shard_bf[MA:MA + MB].T)
        xr = np.ascontiguousarray(shard_bf[MA + MB:].T)
        in_maps.append({"wxa": wxa, "wxb": wxb, "xr": xr, "bb": bb})
    return in_maps


def kernel(x: np.ndarray, W: np.ndarray, b: np.ndarray) -> np.ndarray:
    in_maps = make_in_maps(x, W, b)
    res = _run(in_maps, trace=bool(int(os.environ.get("BASS_KERNEL_TRACE", "0"))))
    y = np.concatenate([res.results[c]["y"] for c in range(N_CORES)], axis=0)
    return y.reshape(B, S, F)
